# revision 12
# baseline (speedup 1.0000x reference)
"""Trainium2 Bass kernel for nn_CT_StochasticGraphDS (gnn_message_passing).

Strategy (8 NeuronCores, SPMD):
  - Row-shard the graph operator A over nodes: core c owns nodes
    [c*512, (c+1)*512). A^T shard (8 MiB fp32) stays resident in SBUF.
  - State h kept in transposed layout hT [(b*8+d), local_node] = [64, 512]
    per core; the full node-major state H [4096, 64] is rebuilt each Euler
    substep via an 8-core AllGather of each core's transposed own-block.
  - A@h computed as hg^T = H^T-chunks (lhsT) x A^T-chunks (rhs) on TensorE
    (fp32r, col-group-paired so the two 64-wide halves run concurrently).
  - All MLPs run in the transposed layout with batch-paired matmuls
    (2 batches per MM via block-diagonal weights), hidden activations on
    ScalarE as single [128, 2048] instructions.
  - times/dt/sqrt(dt)/0.1 folded on the host into per-substep biases,
    scaled W2/z tensors, so no scalar math on device.

Self-contained: hardcodes all shapes; no file I/O.
"""
import numpy as np

N, D, H, IN, RD = 4096, 8, 64, 16, 32
B, S, K = 8, 16, 4
NCORES = 8
NS = N // NCORES          # 512 nodes per core
import os
NSTEP_ENV = int(os.environ.get("KERNEL_NSTEP", "15"))
NSUB_ENV = int(os.environ.get("KERNEL_NSUB", "4"))
NOAG_ENV = int(os.environ.get("KERNEL_NOAG", "0"))
SKIP_PERT_ENV = int(os.environ.get("KERNEL_SKIP_PERT", "0"))
SKIP_TR_ENV = int(os.environ.get("KERNEL_SKIP_TR", "0"))
NSTEP = S - 1             # 15
SK = NSTEP * K            # 60 substeps
NKC = N // 128            # 32 contraction chunks for A@h

_CACHE = {}

LAST_EXEC_NS = None


# ------------------------------------------------------------------ helpers
def _blockdiag2(w):
    """w [8,64] -> [16,128] block-diag for a batch pair."""
    out = np.zeros((16, 128), np.float32)
    out[0:8, 0:64] = w
    out[8:16, 64:128] = w
    return out


def _lohi(w):
    """w [8,64] -> (lo, hi) [64,128] tiles so that slicing at base 0/32
    gives the pair-p weights for rhs slices hT[0:32]/hT[32:64]."""
    bd = _blockdiag2(w)
    lo = np.zeros((64, 128), np.float32)
    lo[0:16] = bd
    lo[32:48] = bd
    hi = np.zeros((64, 128), np.float32)
    hi[16:32] = bd
    hi[48:64] = bd
    return lo, hi


def _scatter_w2(w2):
    """w2 [64, M2] -> [128, 4*64] per-pair zero-padded scatter lhsT tiles.
    Pair p tile at cols [p*64,(p+1)*64): rows 0:64 (even-batch hidden) ->
    out cols 16p..16p+8, rows 64:128 -> 16p+8..16p+16. M2 must be 8."""
    out = np.zeros((128, 256), np.float32)
    for p in range(4):
        out[0:64, p * 64 + 16 * p: p * 64 + 16 * p + 8] = w2
        out[64:128, p * 64 + 16 * p + 8: p * 64 + 16 * p + 16] = w2
    return out


def _build_host_inputs(inputs):
    """Pack the full problem inputs into per-core input dicts."""
    f4 = np.float32
    times = np.asarray(inputs["times"], f4)
    x_all = np.asarray(inputs["inputs"], f4)
    h0 = np.asarray(inputs["h0"], f4)
    A = np.asarray(inputs["A"], f4)
    z = np.asarray(inputs["z"], f4)
    dW1, db1 = np.asarray(inputs["drift_W1"], f4), np.asarray(inputs["drift_b1"], f4)
    dW2, db2 = np.asarray(inputs["drift_W2"], f4), np.asarray(inputs["drift_b2"], f4)
    fW1, fb1 = np.asarray(inputs["diff_W1"], f4), np.asarray(inputs["diff_b1"], f4)
    fW2, fb2 = np.asarray(inputs["diff_W2"], f4), np.asarray(inputs["diff_b2"], f4)
    pW1, pb1 = np.asarray(inputs["pert_W1"], f4), np.asarray(inputs["pert_b1"], f4)
    pW2, pb2 = np.asarray(inputs["pert_W2"], f4), np.asarray(inputs["pert_b2"], f4)
    rW1, rb1 = np.asarray(inputs["read_W1"], f4), np.asarray(inputs["read_b1"], f4)
    rW2, rb2 = np.asarray(inputs["read_W2"], f4), np.asarray(inputs["read_b2"], f4)

    dt = ((times[1:] - times[:-1]) / K).astype(f4)          # [15]
    sqdt = np.sqrt(dt).astype(f4)

    # per-substep m1 biases (t folded in), duplicated for the pair layout
    b1d = np.zeros((128, SK), f4)
    b1f = np.zeros((128, SK), f4)
    for s in range(NSTEP):
        for k in range(K):
            t = f4(times[s] + k * dt[s])
            v = (db1 + t * dW1[8]).astype(f4)
            b1d[0:64, s * 4 + k] = v
            b1d[64:128, s * 4 + k] = v
            v = (fb1 + t * fW1[8]).astype(f4)
            b1f[0:64, s * 4 + k] = v
            b1f[64:128, s * 4 + k] = v

    # drift m2 weights (dt folded), per step and pair: [128, 15*4*64]
    w2d = np.zeros((128, NSTEP * 256), f4)
    for s in range(NSTEP):
        w2d[:, s * 256:(s + 1) * 256] = _scatter_w2(dW2 * dt[s])
    w2f = _scatter_w2(fW2)
    w2p = _scatter_w2(pW2)
    w2r = np.zeros((128, 64), f4)
    w2r[0:64, 0:32] = rW2
    w2r[64:128, 32:64] = rW2

    b2d = np.tile(db2, 8)[:, None] * dt[None, :]            # [64, 15]
    b2d = b2d.astype(f4)
    b2f = np.tile(fb2, 8).astype(f4)[:, None]               # [64, 1]
    b2p = np.tile(pb2, 8).astype(f4)[:, None]
    b1p = np.concatenate([pb1, pb1]).astype(f4)[:, None]    # [128, 1]
    b1r = np.concatenate([rb1, rb1]).astype(f4)[:, None]
    b2r = np.tile(rb2, 2).astype(f4)[:, None]               # [64, 1]

    w1da, w1db = _lohi(dW1[:8])
    w1fa, w1fb = _lohi(fW1[:8])
    w1ra, w1rb = _lohi(rW1)
    # pert weights: uniform K=64 MMs, zero-padded per pair/half
    bdx = np.zeros((32, 128), f4)
    bdx[0:16, 0:64] = pW1[:IN]
    bdx[16:32, 64:128] = pW1[:IN]
    w1xa = np.zeros((64, 128), f4)   # x-part, pairs 0/2 (rows 0:32 of xa/xb)
    w1xa[0:32] = bdx
    w1xb = np.zeros((64, 128), f4)   # x-part, pairs 1/3 (rows 32:64)
    w1xb[32:64] = bdx
    bdh = _blockdiag2(pW1[IN:])
    w1p4 = np.zeros((4, 64, 128), f4)  # h-part, pair p at rows 16p:16p+16
    for p in range(4):
        w1p4[p, 16 * p:16 * p + 16] = bdh

    idn = np.eye(128, dtype=f4)

    # big tensors, transposed/prescaled on host
    hT_g = np.ascontiguousarray(h0.transpose(0, 2, 1).reshape(B * D, N))
    xT_g = x_all[:NSTEP].transpose(0, 1, 3, 2)              # [15, B, IN, N]
    zs_g = (z * (0.1 * sqdt)[:, None, None, None, None]).transpose(0, 1, 2, 4, 3)
    zs_g = np.ascontiguousarray(zs_g).reshape(SK, B * D, N)  # [60, 64, N]

    shared = {
        "B1D": b1d, "B1F": b1f, "W2D": w2d, "W2F": w2f, "W2P": w2p,
        "W2R": w2r, "B2D": b2d, "B2F": b2f, "B2P": b2p, "B1P": b1p,
        "B1R": b1r, "B2R": b2r, "W1DA": w1da, "W1DB": w1db, "W1FA": w1fa,
        "W1FB": w1fb, "W1RA": w1ra, "W1RB": w1rb, "W1XA": w1xa,
        "W1XB": w1xb, "W1P0": w1p4[0], "W1P1": w1p4[1],
        "W1P2": w1p4[2], "W1P3": w1p4[3], "IDN": idn,
    }

    in_maps = []
    for c in range(NCORES):
        sh = slice(c * NS, (c + 1) * NS)
        at = np.ascontiguousarray(A[sh, :].T)               # [4096, 512]
        at = np.ascontiguousarray(
            at.reshape(NKC, 128, NS).transpose(1, 0, 2).reshape(128, NKC * NS))
        xa = np.ascontiguousarray(
            xT_g[:, 0:4, :, sh].reshape(NSTEP, 64, NS))
        xb = np.ascontiguousarray(
            xT_g[:, 4:8, :, sh].reshape(NSTEP, 64, NS))
        m = {
            "AT": at,
            "H0T": np.ascontiguousarray(hT_g[:, sh]),
            "XTA": xa,
            "XTB": xb,
            "ZS": np.ascontiguousarray(zs_g[:, :, sh]),
        }
        m.update(shared)
        in_maps.append(m)
    return in_maps, h0


# ------------------------------------------------------------------ device
def _build_nc():
    import concourse.bacc as bacc
    import concourse.tile as tile
    import concourse.mybir as mybir

    f32 = mybir.dt.float32
    f32r = mybir.dt.float32r
    AF = mybir.ActivationFunctionType
    OP = mybir.AluOpType

    nc = bacc.Bacc("TRN2", target_bir_lowering=False, debug=False,
                   num_devices=NCORES)

    # ---- DRAM I/O  (everything that feeds a matmul is float32r)
    DIN_R = [
        ("AT", [128, NKC * NS]), ("H0T", [B * D, NS]),
        ("XTA", [NSTEP, 64, NS]), ("XTB", [NSTEP, 64, NS]),
        ("W2D", [128, NSTEP * 256]), ("W2F", [128, 256]), ("W2P", [128, 256]),
        ("W2R", [128, 64]),
        ("W1DA", [64, 128]), ("W1DB", [64, 128]), ("W1FA", [64, 128]),
        ("W1FB", [64, 128]), ("W1RA", [64, 128]), ("W1RB", [64, 128]),
        ("W1XA", [64, 128]), ("W1XB", [64, 128]), ("W1P0", [64, 128]),
        ("W1P1", [64, 128]), ("W1P2", [64, 128]), ("W1P3", [64, 128]),
        ("IDN", [128, 128]),
    ]
    DIN_F = [
        ("ZS", [SK, B * D, NS]),
        ("B1D", [128, SK]), ("B1F", [128, SK]),
        ("B2D", [64, NSTEP]), ("B2F", [64, 1]),
        ("B2P", [64, 1]), ("B1P", [128, 1]), ("B1R", [128, 1]),
        ("B2R", [64, 1]),
    ]
    din = {}
    for name, shape in DIN_R:
        din[name] = nc.dram_tensor(name, shape, f32r, kind="ExternalInput")
    for name, shape in DIN_F:
        din[name] = nc.dram_tensor(name, shape, f32, kind="ExternalInput")
    trajT = nc.dram_tensor("trajT", [S, B * D, NS], f32r, kind="ExternalOutput")
    roT = nc.dram_tensor("roT", [S, B * RD, NS], f32, kind="ExternalOutput")

    rg = [list(range(NCORES))]

    with tile.TileContext(nc) as tc:
        with tc.tile_pool(name="const", bufs=1) as constp, \
             tc.tile_pool(name="state", bufs=3) as statep, \
             tc.tile_pool(name="hfp", bufs=2) as hfp, \
             tc.tile_pool(name="work", bufs=2) as workp, \
             tc.tile_pool(name="small", bufs=3) as smallp, \
             tc.tile_pool(name="zxp", bufs=3) as zxp, \
             tc.tile_pool(name="ps_y1", bufs=1, space="PSUM") as ps_y1, \
             tc.tile_pool(name="ps_ah", bufs=2, space="PSUM") as ps_ah, \
             tc.tile_pool(name="ps_m2", bufs=1, space="PSUM") as ps_m2, \
             tc.tile_pool(name="ps_tr", bufs=1, space="PSUM") as ps_tr, \
             tc.tile_pool(name="dram", bufs=1, space="DRAM") as dramp:

            # ---- persistent SBUF
            at_sb = constp.tile([128, NKC * NS], f32r, name="at_sb")
            nc.sync.dma_start(at_sb[:], din["AT"][:])
            cw = {}
            for name, shape in DIN_R[4:] + DIN_F[1:]:
                dt_ = f32r if any(name == n for n, _ in DIN_R) else f32
                t = constp.tile(shape, dt_, name=f"c_{name}")
                nc.sync.dma_start(t[:], din[name][:])
                cw[name] = t

            ag_ctr = [0]

            # ---- helpers ------------------------------------------------
            def m1_mms(y1, rhs, wa, wb):
                for p in range(4):
                    b0 = 32 * (p // 2)
                    wt = wa if p % 2 == 0 else wb
                    nc.tensor.matmul(
                        y1[:, p * NS:(p + 1) * NS],
                        lhsT=wt[b0:b0 + 32, :],
                        rhs=rhs[b0:b0 + 32, :],
                        start=True, stop=True)

            def m2_mms(dst, w2t, off, y1sb):
                for p in range(4):
                    nc.tensor.matmul(
                        dst,
                        lhsT=w2t[:, off + p * 64:off + (p + 1) * 64],
                        rhs=y1sb[:, p * NS:(p + 1) * NS],
                        start=(p == 0), stop=(p == 3))

            def emit_ag(hT_t):
                """transpose own block -> allgather -> fresh H_full tile"""
                if SKIP_TR_ENV:
                    hf = hfp.tile([128, NKC * 64], f32r, name="hf")
                    nc.sync.dma_start(hf[:], din["AT"][:, 0:NKC * 64])
                    return hf
                i = ag_ctr[0]
                ag_ctr[0] += 1
                ag_in = dramp.tile([128, 256], f32r, name=f"ag_in{i}",
                                   tag=f"agi{i}")
                ag_out = dramp.tile([NCORES, 128, 256], f32r,
                                    addr_space="Shared", name=f"ag_out{i}",
                                    tag=f"ago{i}")
                tr = ps_tr.tile([128, 256], f32r, name="tr")
                for cc in range(4):
                    nc.tensor.transpose(
                        tr[:, cc * 64:(cc + 1) * 64],
                        hT_t[:, cc * 128:(cc + 1) * 128],
                        cw["IDN"][0:64, 0:64])
                hown = smallp.tile([128, 256], f32r, name="hown")
                nc.vector.tensor_copy(hown[:], tr[:])
                nc.sync.dma_start(ag_in[:], hown[:])
                if NOAG_ENV:
                    hf = hfp.tile([128, NKC * 64], f32r, name="hf")
                    for r in range(NCORES):
                        nc.sync.dma_start(hf[:, r * 256:(r + 1) * 256],
                                          ag_in[:])
                    return hf
                nc.gpsimd.collective_compute(
                    "AllGather", mybir.AluOpType.bypass, replica_groups=rg,
                    ins=[ag_in.opt()], outs=[ag_out.opt()])
                hf = hfp.tile([128, NKC * 64], f32r, name="hf")
                for r in range(NCORES):
                    nc.sync.dma_start(hf[:, r * 256:(r + 1) * 256], ag_out[r])
                return hf

            def emit_ah(hf):
                """hg^T [64,512] = (A @ h) in transposed layout"""
                ah = ps_ah.tile([64, NS], f32, name="ah", tag="ahx")
                for kc in range(NKC):
                    nc.tensor.matmul(
                        ah[:, :],
                        lhsT=hf[:, kc * 64:(kc + 1) * 64],
                        rhs=at_sb[:, kc * NS:(kc + 1) * NS],
                        start=(kc == 0), stop=(kc == NKC - 1))
                hg = workp.tile([64, NS], f32r, name="hg")
                nc.vector.tensor_copy(hg[:], ah[:])
                return hg

            def emit_readout(hT_t, s_out):
                y1 = ps_y1.tile([128, 4 * NS], f32, name="y1")
                m1_mms(y1, hT_t, cw["W1RA"], cw["W1RB"])
                y1sb = workp.tile([128, 4 * NS], f32r, name="y1sb")
                nc.vector.tensor_scalar(
                    y1sb[:], y1[:], cw["B1R"][:, 0:1], 0.0,
                    op0=OP.add, op1=OP.max)
                for p in range(4):
                    rop = ps_m2.tile([64, NS], f32, name="m2c", tag="m2c")
                    nc.tensor.matmul(
                        rop[:, :],
                        lhsT=cw["W2R"][:],
                        rhs=y1sb[:, p * NS:(p + 1) * NS],
                        start=True, stop=True)
                    rosb = smallp.tile([64, NS], f32, name="rosb")
                    nc.scalar.activation(rosb[:], rop[:], AF.Identity,
                                         bias=cw["B2R"][:, 0:1])
                    nc.scalar.dma_start(
                        roT[s_out, p * 64:(p + 1) * 64, :], rosb[:])

            # ---- init ---------------------------------------------------
            hT = statep.tile([B * D, NS], f32r, name="hT")
            nc.sync.dma_start(hT[:], din["H0T"][:])
            hT_out = hT  # state pending readout/traj output

            for s in range(min(NSTEP, NSTEP_ENV)):
                # -- load step inputs
                xa = zxp.tile([64, NS], f32r, name="xa")
                xb = zxp.tile([64, NS], f32r, name="xb")
                nc.scalar.dma_start(xa[:], din["XTA"][s])
                nc.scalar.dma_start(xb[:], din["XTB"][s])

                # -- perturbation
                if SKIP_PERT_ENV:
                    hf = emit_ag(hT)
                    emit_readout(hT_out, s)
                    if s > 0:
                        nc.scalar.dma_start(trajT[s], hT_out[:])
                    for k in range(min(K, NSUB_ENV)):
                        pass
                    hT_out = hT
                    continue
                y1 = ps_y1.tile([128, 4 * NS], f32, name="y1")
                for p in range(4):
                    xt = xa if p < 2 else xb
                    xw = cw["W1XA"] if p % 2 == 0 else cw["W1XB"]
                    nc.tensor.matmul(
                        y1[:, p * NS:(p + 1) * NS],
                        lhsT=xw[:, :], rhs=xt[:, :],
                        start=True, stop=False)
                    nc.tensor.matmul(
                        y1[:, p * NS:(p + 1) * NS],
                        lhsT=cw[f"W1P{p}"][:, :], rhs=hT[:, :],
                        start=False, stop=True)
                y1sb = workp.tile([128, 4 * NS], f32r, name="y1sb")
                nc.scalar.activation(y1sb[:], y1[:], AF.Relu,
                                     bias=cw["B1P"][:, 0:1])
                m2c = ps_m2.tile([64, NS], f32, name="m2c", tag="m2c")
                m2_mms(m2c[:, :], cw["W2P"], 0, y1sb)
                dl = smallp.tile([64, NS], f32, name="dl")
                nc.vector.tensor_scalar_add(dl[:], m2c[:, :],
                                            cw["B2P"][:, 0:1])
                hT_new = statep.tile([B * D, NS], f32r, name="hT")
                nc.vector.tensor_add(hT_new[:], hT[:], dl[:])
                hT = hT_new

                hf = emit_ag(hT)

                # readout + traj write of the PREVIOUS boundary state --
                # fills the collective shadow
                emit_readout(hT_out, s)
                if s > 0:
                    nc.scalar.dma_start(trajT[s], hT_out[:])

                for k in range(min(K, NSUB_ENV)):
                    sk = s * 4 + k
                    zst = zxp.tile([64, NS], f32, name="zst")
                    nc.scalar.dma_start(zst[:], din["ZS"][sk])

                    # diffusion (depends only on hT -> runs in AG shadow)
                    y1f = ps_y1.tile([128, 4 * NS], f32, name="y1")
                    m1_mms(y1f, hT, cw["W1FA"], cw["W1FB"])
                    y1fsb = workp.tile([128, 4 * NS], f32r, name="y1sb")
                    nc.scalar.activation(y1fsb[:], y1f[:], AF.Sigmoid,
                                         bias=cw["B1F"][:, sk:sk + 1])
                    dfm2 = ps_ah.tile([64, NS], f32, name="dfm2", tag="ahx")
                    m2_mms(dfm2[:, :], cw["W2F"], 0, y1fsb)
                    sg = smallp.tile([64, NS], f32, name="sg")
                    nc.scalar.activation(sg[:], dfm2[:, :], AF.Sigmoid,
                                         bias=cw["B2F"][:, 0:1])
                    noise = smallp.tile([64, NS], f32, name="noise")
                    nc.vector.tensor_mul(noise[:], sg[:], zst[:])
                    u = smallp.tile([64, NS], f32, name="u")
                    nc.vector.tensor_add(u[:], hT[:], noise[:])

                    # A @ h
                    hg = emit_ah(hf)

                    # drift
                    y1d = ps_y1.tile([128, 4 * NS], f32, name="y1")
                    m1_mms(y1d, hg, cw["W1DA"], cw["W1DB"])
                    y1dsb = workp.tile([128, 4 * NS], f32r, name="y1sb")
                    nc.scalar.activation(y1dsb[:], y1d[:], AF.Tanh,
                                         bias=cw["B1D"][:, sk:sk + 1])
                    m2c = ps_m2.tile([64, NS], f32, name="m2c", tag="m2c")
                    m2_mms(m2c[:, :], cw["W2D"], s * 256, y1dsb)
                    drs = smallp.tile([64, NS], f32, name="drs")
                    nc.vector.tensor_scalar_add(drs[:], m2c[:, :],
                                                cw["B2D"][:, s:s + 1])
                    hT_new = statep.tile([B * D, NS], f32r, name="hT")
                    nc.vector.tensor_add(hT_new[:], u[:], drs[:])
                    hT = hT_new

                    if k < K - 1:
                        hf = emit_ag(hT)

                hT_out = hT

            # final boundary outputs
            emit_readout(hT_out, min(NSTEP, NSTEP_ENV))
            nc.scalar.dma_start(trajT[min(NSTEP, NSTEP_ENV)], hT_out[:])

    nc.compile()
    return nc


def kernel(**inputs):
    global LAST_EXEC_NS
    from concourse.bass_utils import run_bass_kernel_spmd

    if "nc" not in _CACHE:
        _CACHE["nc"] = _build_nc()
    nc = _CACHE["nc"]

    in_maps, h0 = _build_host_inputs(inputs)
    import os as _os
    _trace = bool(int(_os.environ.get("KERNEL_TRACE", "0")))
    res = run_bass_kernel_spmd(nc, in_maps, core_ids=list(range(NCORES)),
                               trace=_trace)
    LAST_EXEC_NS = res.exec_time_ns
    if _trace:
        _CACHE["last_res"] = res

    f4 = np.float32
    traj = np.empty((S, B, N, D), f4)
    ro = np.empty((S, B, N, RD), f4)
    traj[0] = np.asarray(inputs["h0"], f4)
    for c in range(NCORES):
        sh = slice(c * NS, (c + 1) * NS)
        tT = res.results[c]["trajT"]          # [16, 64, 512]
        traj[1:, :, sh, :] = tT[1:].reshape(NSTEP, B, D, NS).transpose(0, 1, 3, 2)
        ro[:, :, sh, :] = res.results[c]["roT"].reshape(
            S, B, RD, NS).transpose(0, 1, 3, 2)
    return traj, ro


# revision 14
# speedup vs baseline: 1.3073x; 1.3073x over previous
"""Trainium2 Bass kernel for nn_CT_StochasticGraphDS (gnn_message_passing).

Strategy (8 NeuronCores, SPMD):
  - Row-shard the graph operator A over nodes: core c owns nodes
    [c*512, (c+1)*512). A^T shard (8 MiB fp32) stays resident in SBUF.
  - State h kept in transposed layout hT [(b*8+d), local_node] = [64, 512]
    per core; the full node-major state H [4096, 64] is rebuilt each Euler
    substep via an 8-core AllGather of each core's transposed own-block.
  - A@h computed as hg^T = H^T-chunks (lhsT) x A^T-chunks (rhs) on TensorE
    (fp32r, col-group-paired so the two 64-wide halves run concurrently).
  - All MLPs run in the transposed layout with batch-paired matmuls
    (2 batches per MM via block-diagonal weights), hidden activations on
    ScalarE as single [128, 2048] instructions.
  - times/dt/sqrt(dt)/0.1 folded on the host into per-substep biases,
    scaled W2/z tensors, so no scalar math on device.

Self-contained: hardcodes all shapes; no file I/O.
"""
import numpy as np

N, D, H, IN, RD = 4096, 8, 64, 16, 32
B, S, K = 8, 16, 4
NCORES = 8
NS = N // NCORES          # 512 nodes per core
import os
NSTEP_ENV = int(os.environ.get("KERNEL_NSTEP", "15"))
NSUB_ENV = int(os.environ.get("KERNEL_NSUB", "4"))
NOAG_ENV = int(os.environ.get("KERNEL_NOAG", "0"))
SKIP_PERT_ENV = int(os.environ.get("KERNEL_SKIP_PERT", "0"))
SKIP_TR_ENV = int(os.environ.get("KERNEL_SKIP_TR", "0"))
NSTEP = S - 1             # 15
SK = NSTEP * K            # 60 substeps
NKC = N // 128            # 32 contraction chunks for A@h

_CACHE = {}

LAST_EXEC_NS = None


# ------------------------------------------------------------------ helpers
def _blockdiag2(w):
    """w [8,64] -> [16,128] block-diag for a batch pair."""
    out = np.zeros((16, 128), np.float32)
    out[0:8, 0:64] = w
    out[8:16, 64:128] = w
    return out


def _lohi(w):
    """w [8,64] -> (lo, hi) [64,128] tiles so that slicing at base 0/32
    gives the pair-p weights for rhs slices hT[0:32]/hT[32:64]."""
    bd = _blockdiag2(w)
    lo = np.zeros((64, 128), np.float32)
    lo[0:16] = bd
    lo[32:48] = bd
    hi = np.zeros((64, 128), np.float32)
    hi[16:32] = bd
    hi[48:64] = bd
    return lo, hi


def _scatter_w2(w2):
    """w2 [64, M2] -> [128, 4*64] per-pair zero-padded scatter lhsT tiles.
    Pair p tile at cols [p*64,(p+1)*64): rows 0:64 (even-batch hidden) ->
    out cols 16p..16p+8, rows 64:128 -> 16p+8..16p+16. M2 must be 8."""
    out = np.zeros((128, 256), np.float32)
    for p in range(4):
        out[0:64, p * 64 + 16 * p: p * 64 + 16 * p + 8] = w2
        out[64:128, p * 64 + 16 * p + 8: p * 64 + 16 * p + 16] = w2
    return out


def _build_host_inputs(inputs):
    """Pack the full problem inputs into per-core input dicts."""
    f4 = np.float32
    times = np.asarray(inputs["times"], f4)
    x_all = np.asarray(inputs["inputs"], f4)
    h0 = np.asarray(inputs["h0"], f4)
    A = np.asarray(inputs["A"], f4)
    z = np.asarray(inputs["z"], f4)
    dW1, db1 = np.asarray(inputs["drift_W1"], f4), np.asarray(inputs["drift_b1"], f4)
    dW2, db2 = np.asarray(inputs["drift_W2"], f4), np.asarray(inputs["drift_b2"], f4)
    fW1, fb1 = np.asarray(inputs["diff_W1"], f4), np.asarray(inputs["diff_b1"], f4)
    fW2, fb2 = np.asarray(inputs["diff_W2"], f4), np.asarray(inputs["diff_b2"], f4)
    pW1, pb1 = np.asarray(inputs["pert_W1"], f4), np.asarray(inputs["pert_b1"], f4)
    pW2, pb2 = np.asarray(inputs["pert_W2"], f4), np.asarray(inputs["pert_b2"], f4)
    rW1, rb1 = np.asarray(inputs["read_W1"], f4), np.asarray(inputs["read_b1"], f4)
    rW2, rb2 = np.asarray(inputs["read_W2"], f4), np.asarray(inputs["read_b2"], f4)

    dt = ((times[1:] - times[:-1]) / K).astype(f4)          # [15]
    sqdt = np.sqrt(dt).astype(f4)

    # per-substep m1 biases (t folded in), duplicated for the pair layout
    b1d = np.zeros((128, SK), f4)
    b1f = np.zeros((128, SK), f4)
    for s in range(NSTEP):
        for k in range(K):
            t = f4(times[s] + k * dt[s])
            v = (db1 + t * dW1[8]).astype(f4)
            b1d[0:64, s * 4 + k] = v
            b1d[64:128, s * 4 + k] = v
            v = (fb1 + t * fW1[8]).astype(f4)
            b1f[0:64, s * 4 + k] = v
            b1f[64:128, s * 4 + k] = v

    # drift m2 weights (dt folded), per step and pair: [128, 15*4*64]
    w2d = np.zeros((128, NSTEP * 256), f4)
    for s in range(NSTEP):
        w2d[:, s * 256:(s + 1) * 256] = _scatter_w2(dW2 * dt[s])
    w2f = _scatter_w2(fW2)
    w2p = _scatter_w2(pW2)
    w2r = np.zeros((128, 64), f4)
    w2r[0:64, 0:32] = rW2
    w2r[64:128, 32:64] = rW2

    b2d = np.tile(db2, 8)[:, None] * dt[None, :]            # [64, 15]
    b2d = b2d.astype(f4)
    b2f = np.tile(fb2, 8).astype(f4)[:, None]               # [64, 1]
    b2p = np.tile(pb2, 8).astype(f4)[:, None]
    b1p = np.concatenate([pb1, pb1]).astype(f4)[:, None]    # [128, 1]
    b1r = np.concatenate([rb1, rb1]).astype(f4)[:, None]
    b2r = np.tile(rb2, 2).astype(f4)[:, None]               # [64, 1]

    w1da, w1db = _lohi(dW1[:8])
    w1fa, w1fb = _lohi(fW1[:8])
    w1ra, w1rb = _lohi(rW1)
    # pert weights: uniform K=64 MMs, zero-padded per pair/half
    bdx = np.zeros((32, 128), f4)
    bdx[0:16, 0:64] = pW1[:IN]
    bdx[16:32, 64:128] = pW1[:IN]
    w1xa = np.zeros((64, 128), f4)   # x-part, pairs 0/2 (rows 0:32 of xa/xb)
    w1xa[0:32] = bdx
    w1xb = np.zeros((64, 128), f4)   # x-part, pairs 1/3 (rows 32:64)
    w1xb[32:64] = bdx
    bdh = _blockdiag2(pW1[IN:])
    w1p4 = np.zeros((4, 64, 128), f4)  # h-part, pair p at rows 16p:16p+16
    for p in range(4):
        w1p4[p, 16 * p:16 * p + 16] = bdh

    idn = np.eye(128, dtype=f4)

    # big tensors, transposed/prescaled on host
    hT_g = np.ascontiguousarray(h0.transpose(0, 2, 1).reshape(B * D, N))
    xT_g = x_all[:NSTEP].transpose(0, 1, 3, 2)              # [15, B, IN, N]
    zs_g = (z * (0.1 * sqdt)[:, None, None, None, None]).transpose(0, 1, 2, 4, 3)
    zs_g = np.ascontiguousarray(zs_g).reshape(SK, B * D, N)  # [60, 64, N]

    shared = {
        "B1D": b1d, "B1F": b1f, "W2D": w2d, "W2F": w2f, "W2P": w2p,
        "W2R": w2r, "B2D": b2d, "B2F": b2f, "B2P": b2p, "B1P": b1p,
        "B1R": b1r, "B2R": b2r, "W1DA": w1da, "W1DB": w1db, "W1FA": w1fa,
        "W1FB": w1fb, "W1RA": w1ra, "W1RB": w1rb, "W1XA": w1xa,
        "W1XB": w1xb, "W1P0": w1p4[0], "W1P1": w1p4[1],
        "W1P2": w1p4[2], "W1P3": w1p4[3], "IDN": idn,
    }

    in_maps = []
    for c in range(NCORES):
        sh = slice(c * NS, (c + 1) * NS)
        at = np.ascontiguousarray(A[sh, :].T) * np.float32(4096.0)
        at = np.ascontiguousarray(
            at.reshape(NKC, 128, NS).transpose(1, 0, 2).reshape(
                128, NKC * NS)).astype(np.float16)
        xa = np.ascontiguousarray(
            xT_g[:, 0:4, :, sh].reshape(NSTEP, 64, NS))
        xb = np.ascontiguousarray(
            xT_g[:, 4:8, :, sh].reshape(NSTEP, 64, NS))
        m = {
            "AT": at,
            "H0T": np.ascontiguousarray(hT_g[:, sh]),
            "XTA": xa,
            "XTB": xb,
            "ZS": np.ascontiguousarray(zs_g[:, :, sh]),
        }
        m.update(shared)
        in_maps.append(m)
    return in_maps, h0


# ------------------------------------------------------------------ device
def _build_nc():
    import concourse.bacc as bacc
    import concourse.tile as tile
    import concourse.mybir as mybir

    f32 = mybir.dt.float32
    f32r = mybir.dt.float32r
    AF = mybir.ActivationFunctionType
    OP = mybir.AluOpType

    nc = bacc.Bacc("TRN2", target_bir_lowering=False, debug=False,
                   num_devices=NCORES)

    # ---- DRAM I/O  (everything that feeds a matmul is float32r)
    f16 = mybir.dt.float16
    DIN_16 = [("AT", [128, NKC * NS])]
    DIN_R = [
        ("H0T", [B * D, NS]),
        ("XTA", [NSTEP, 64, NS]), ("XTB", [NSTEP, 64, NS]),
        ("W2D", [128, NSTEP * 256]), ("W2F", [128, 256]), ("W2P", [128, 256]),
        ("W2R", [128, 64]),
        ("W1DA", [64, 128]), ("W1DB", [64, 128]), ("W1FA", [64, 128]),
        ("W1FB", [64, 128]), ("W1RA", [64, 128]), ("W1RB", [64, 128]),
        ("W1XA", [64, 128]), ("W1XB", [64, 128]), ("W1P0", [64, 128]),
        ("W1P1", [64, 128]), ("W1P2", [64, 128]), ("W1P3", [64, 128]),
        ("IDN", [128, 128]),
    ]
    DIN_F = [
        ("ZS", [SK, B * D, NS]),
        ("B1D", [128, SK]), ("B1F", [128, SK]),
        ("B2D", [64, NSTEP]), ("B2F", [64, 1]),
        ("B2P", [64, 1]), ("B1P", [128, 1]), ("B1R", [128, 1]),
        ("B2R", [64, 1]),
    ]
    din = {}
    for name, shape in DIN_16:
        din[name] = nc.dram_tensor(name, shape, f16, kind="ExternalInput")
    for name, shape in DIN_R:
        din[name] = nc.dram_tensor(name, shape, f32r, kind="ExternalInput")
    for name, shape in DIN_F:
        din[name] = nc.dram_tensor(name, shape, f32, kind="ExternalInput")
    trajT = nc.dram_tensor("trajT", [S, B * D, NS], f32r, kind="ExternalOutput")
    roT = nc.dram_tensor("roT", [S, B * RD, NS], f32, kind="ExternalOutput")

    rg = [list(range(NCORES))]

    with tile.TileContext(nc) as tc:
        with tc.tile_pool(name="const", bufs=1) as constp, \
             tc.tile_pool(name="state", bufs=3) as statep, \
             tc.tile_pool(name="hfp", bufs=2) as hfp, \
             tc.tile_pool(name="work", bufs=2) as workp, \
             tc.tile_pool(name="small", bufs=3) as smallp, \
             tc.tile_pool(name="zxp", bufs=3) as zxp, \
             tc.tile_pool(name="ps_y1", bufs=1, space="PSUM") as ps_y1, \
             tc.tile_pool(name="ps_ah", bufs=2, space="PSUM") as ps_ah, \
             tc.tile_pool(name="ps_m2", bufs=1, space="PSUM") as ps_m2, \
             tc.tile_pool(name="ps_tr", bufs=1, space="PSUM") as ps_tr, \
             tc.tile_pool(name="dram", bufs=1, space="DRAM") as dramp:

            # ---- persistent SBUF
            at_sb = constp.tile([128, NKC * NS], f16, name="at_sb")
            nc.sync.dma_start(at_sb[:], din["AT"][:])
            cw = {}
            for name, shape in DIN_R[3:] + DIN_F[1:]:
                dt_ = f32r if any(name == n for n, _ in DIN_R) else f32
                t = constp.tile(shape, dt_, name=f"c_{name}")
                nc.sync.dma_start(t[:], din[name][:])
                cw[name] = t

            ag_ctr = [0]

            # ---- helpers ------------------------------------------------
            def m1_mms(y1, rhs, wa, wb):
                for p in range(4):
                    b0 = 32 * (p // 2)
                    wt = wa if p % 2 == 0 else wb
                    nc.tensor.matmul(
                        y1[:, p * NS:(p + 1) * NS],
                        lhsT=wt[b0:b0 + 32, :],
                        rhs=rhs[b0:b0 + 32, :],
                        start=True, stop=True)

            def m2_mms(dst, w2t, off, y1sb):
                for p in range(4):
                    nc.tensor.matmul(
                        dst,
                        lhsT=w2t[:, off + p * 64:off + (p + 1) * 64],
                        rhs=y1sb[:, p * NS:(p + 1) * NS],
                        start=(p == 0), stop=(p == 3))

            def emit_ag(hT_t):
                """transpose own block -> allgather -> fresh H_full tile"""
                if SKIP_TR_ENV:
                    hf = hfp.tile([128, NKC * 64], f32r, name="hf")
                    nc.sync.dma_start(hf[:], din["AT"][:, 0:NKC * 64])
                    return hf
                i = ag_ctr[0]
                ag_ctr[0] += 1
                ag_in = dramp.tile([128, 256], f16, name=f"ag_in{i}",
                                   tag=f"agi{i}")
                ag_out = dramp.tile([NCORES, 128, 256], f16,
                                    addr_space="Shared", name=f"ag_out{i}",
                                    tag=f"ago{i}")
                tr = ps_tr.tile([128, 256], f32r, name="tr")
                for cc in range(4):
                    nc.tensor.transpose(
                        tr[:, cc * 64:(cc + 1) * 64],
                        hT_t[:, cc * 128:(cc + 1) * 128],
                        cw["IDN"][0:64, 0:64])
                hown = smallp.tile([128, 256], f16, name="hown")
                nc.vector.tensor_copy(hown[:], tr[:])
                nc.sync.dma_start(ag_in[:], hown[:])
                if NOAG_ENV:
                    hf = hfp.tile([128, NKC * 64], f16, name="hf")
                    for r in range(NCORES):
                        nc.sync.dma_start(hf[:, r * 256:(r + 1) * 256],
                                          ag_in[:])
                    return hf
                nc.gpsimd.collective_compute(
                    "AllGather", mybir.AluOpType.bypass, replica_groups=rg,
                    ins=[ag_in.opt()], outs=[ag_out.opt()])
                hf = hfp.tile([128, NKC * 64], f16, name="hf")
                for r in range(NCORES):
                    nc.sync.dma_start(hf[:, r * 256:(r + 1) * 256], ag_out[r])
                return hf

            AH_SCALE = float(2.0 ** -12)

            def emit_ah(hf):
                """hg^T [64,512] = (A @ h); A is prescaled by 2^12 in fp16"""
                ah = ps_ah.tile([128, NS], f32, name="ah", tag="ahx")
                for kc in range(NKC):
                    g = kc % 2
                    nc.tensor.matmul(
                        ah[g * 64:(g + 1) * 64, :],
                        lhsT=hf[:, kc * 64:(kc + 1) * 64],
                        rhs=at_sb[:, kc * NS:(kc + 1) * NS],
                        start=(kc < 2), stop=(kc >= NKC - 2),
                        tile_position=(0, g * 64))
                hgb = smallp.tile([64, NS], f32, name="hgb")
                nc.scalar.mul(hgb[:], ah[64:128, :], AH_SCALE)
                hg = workp.tile([64, NS], f32r, name="hg")
                nc.vector.scalar_tensor_tensor(
                    hg[:], ah[0:64, :], AH_SCALE, hgb[:],
                    op0=OP.mult, op1=OP.add)
                return hg

            def emit_readout(hT_t, s_out):
                y1 = ps_y1.tile([128, 4 * NS], f32, name="y1")
                m1_mms(y1, hT_t, cw["W1RA"], cw["W1RB"])
                y1sb = workp.tile([128, 4 * NS], f32r, name="y1sb")
                nc.vector.tensor_scalar(
                    y1sb[:], y1[:], cw["B1R"][:, 0:1], 0.0,
                    op0=OP.add, op1=OP.max)
                for p in range(4):
                    rop = ps_m2.tile([64, NS], f32, name="m2c", tag="m2c")
                    nc.tensor.matmul(
                        rop[:, :],
                        lhsT=cw["W2R"][:],
                        rhs=y1sb[:, p * NS:(p + 1) * NS],
                        start=True, stop=True)
                    rosb = smallp.tile([64, NS], f32, name="rosb")
                    nc.scalar.activation(rosb[:], rop[:], AF.Identity,
                                         bias=cw["B2R"][:, 0:1])
                    nc.scalar.dma_start(
                        roT[s_out, p * 64:(p + 1) * 64, :], rosb[:])

            # ---- init ---------------------------------------------------
            hT = statep.tile([B * D, NS], f32r, name="hT")
            nc.sync.dma_start(hT[:], din["H0T"][:])
            hT_out = hT  # state pending readout/traj output

            for s in range(min(NSTEP, NSTEP_ENV)):
                # -- load step inputs
                xa = zxp.tile([64, NS], f32r, name="xa")
                xb = zxp.tile([64, NS], f32r, name="xb")
                nc.scalar.dma_start(xa[:], din["XTA"][s])
                nc.scalar.dma_start(xb[:], din["XTB"][s])

                # -- perturbation
                if SKIP_PERT_ENV:
                    hf = emit_ag(hT)
                    emit_readout(hT_out, s)
                    if s > 0:
                        nc.scalar.dma_start(trajT[s], hT_out[:])
                    for k in range(min(K, NSUB_ENV)):
                        pass
                    hT_out = hT
                    continue
                y1 = ps_y1.tile([128, 4 * NS], f32, name="y1")
                for p in range(4):
                    xt = xa if p < 2 else xb
                    xw = cw["W1XA"] if p % 2 == 0 else cw["W1XB"]
                    nc.tensor.matmul(
                        y1[:, p * NS:(p + 1) * NS],
                        lhsT=xw[:, :], rhs=xt[:, :],
                        start=True, stop=False)
                    nc.tensor.matmul(
                        y1[:, p * NS:(p + 1) * NS],
                        lhsT=cw[f"W1P{p}"][:, :], rhs=hT[:, :],
                        start=False, stop=True)
                y1sb = workp.tile([128, 4 * NS], f32r, name="y1sb")
                nc.scalar.activation(y1sb[:], y1[:], AF.Relu,
                                     bias=cw["B1P"][:, 0:1])
                m2c = ps_m2.tile([64, NS], f32, name="m2c", tag="m2c")
                m2_mms(m2c[:, :], cw["W2P"], 0, y1sb)
                hT_new = statep.tile([B * D, NS], f32r, name="hT")
                nc.vector.scalar_tensor_tensor(
                    hT_new[:], m2c[:, :], cw["B2P"][:, 0:1], hT[:],
                    op0=OP.add, op1=OP.add)
                hT = hT_new

                hf = emit_ag(hT)

                # readout + traj write of the PREVIOUS boundary state --
                # fills the collective shadow
                emit_readout(hT_out, s)
                if s > 0:
                    nc.scalar.dma_start(trajT[s], hT_out[:])

                for k in range(min(K, NSUB_ENV)):
                    sk = s * 4 + k
                    zst = zxp.tile([64, NS], f32, name="zst")
                    nc.scalar.dma_start(zst[:], din["ZS"][sk])

                    # diffusion (depends only on hT -> runs in AG shadow)
                    y1f = ps_y1.tile([128, 4 * NS], f32, name="y1")
                    m1_mms(y1f, hT, cw["W1FA"], cw["W1FB"])
                    y1fsb = workp.tile([128, 4 * NS], f32r, name="y1sb")
                    nc.scalar.activation(y1fsb[:], y1f[:], AF.Sigmoid,
                                         bias=cw["B1F"][:, sk:sk + 1])
                    dfm2 = ps_ah.tile([64, NS], f32, name="dfm2", tag="ahx")
                    m2_mms(dfm2[:, :], cw["W2F"], 0, y1fsb)
                    sg = smallp.tile([64, NS], f32, name="sg")
                    nc.scalar.activation(sg[:], dfm2[:, :], AF.Sigmoid,
                                         bias=cw["B2F"][:, 0:1])
                    noise = smallp.tile([64, NS], f32, name="noise")
                    nc.vector.tensor_mul(noise[:], sg[:], zst[:])
                    u = smallp.tile([64, NS], f32, name="u")
                    nc.vector.tensor_add(u[:], hT[:], noise[:])

                    # A @ h
                    hg = emit_ah(hf)

                    # drift
                    y1d = ps_y1.tile([128, 4 * NS], f32, name="y1")
                    m1_mms(y1d, hg, cw["W1DA"], cw["W1DB"])
                    y1dsb = workp.tile([128, 4 * NS], f32r, name="y1sb")
                    nc.scalar.activation(y1dsb[:], y1d[:], AF.Tanh,
                                         bias=cw["B1D"][:, sk:sk + 1])
                    m2c = ps_m2.tile([64, NS], f32, name="m2c", tag="m2c")
                    m2_mms(m2c[:, :], cw["W2D"], s * 256, y1dsb)
                    hT_new = statep.tile([B * D, NS], f32r, name="hT")
                    nc.vector.scalar_tensor_tensor(
                        hT_new[:], m2c[:, :], cw["B2D"][:, s:s + 1], u[:],
                        op0=OP.add, op1=OP.add)
                    hT = hT_new

                    if k < K - 1:
                        hf = emit_ag(hT)

                hT_out = hT

            # final boundary outputs
            emit_readout(hT_out, min(NSTEP, NSTEP_ENV))
            nc.scalar.dma_start(trajT[min(NSTEP, NSTEP_ENV)], hT_out[:])

    nc.compile()
    return nc


def kernel(**inputs):
    global LAST_EXEC_NS
    from concourse.bass_utils import run_bass_kernel_spmd

    if "nc" not in _CACHE:
        _CACHE["nc"] = _build_nc()
    nc = _CACHE["nc"]

    in_maps, h0 = _build_host_inputs(inputs)
    import os as _os
    _trace = bool(int(_os.environ.get("KERNEL_TRACE", "0")))
    res = run_bass_kernel_spmd(nc, in_maps, core_ids=list(range(NCORES)),
                               trace=_trace)
    LAST_EXEC_NS = res.exec_time_ns
    if _trace:
        _CACHE["last_res"] = res

    f4 = np.float32
    traj = np.empty((S, B, N, D), f4)
    ro = np.empty((S, B, N, RD), f4)
    traj[0] = np.asarray(inputs["h0"], f4)
    for c in range(NCORES):
        sh = slice(c * NS, (c + 1) * NS)
        tT = res.results[c]["trajT"]          # [16, 64, 512]
        traj[1:, :, sh, :] = tT[1:].reshape(NSTEP, B, D, NS).transpose(0, 1, 3, 2)
        ro[:, :, sh, :] = res.results[c]["roT"].reshape(
            S, B, RD, NS).transpose(0, 1, 3, 2)
    return traj, ro
